# revision 13
# baseline (speedup 1.0000x reference)
"""Trainium2 Bass kernel for the MixedHighwayCell scan problem.

Reference computation (per timestep t, sequential over T=512):
    pre_t = h_{t-1} + alpha*wx_t + beta*(h_{t-1} @ W_h.T)
    h_t   = pre_t * rsqrt(mean(pre_t^2) + eps)
    out_t = h_t * silu(h_t)
with wx = x @ W.T + b precomputed for all t.

Strategy (8 cores, batch-sharded: 4 batch rows per core, no collectives):
  Phase A: uT[d, t, b] = alpha*(x @ W.T + b) computed in exact fp32,
           directly in transposed (d-major) layout, kept in SBUF.
  Phase B: sequential scan with the state kept UNNORMALIZED (pre_t) in
           d-major layout [128, 8, NB].  The rms scale s_t is pushed
           through the recurrent matmul:
               mm_t     = pre_{t-1} @ Wh_lp.T        (low precision OK:
                                                      result is scaled by beta)
               pre_t    = h_{t-1} + u_t + (beta*s_{t-1}) * mm_t
               h_{t-1}  = s_{t-1} * pre_{t-1}
           The matmul streams Wh^T (moving operand) against the thin
           stationary pre^T tiles; its b-major output is transposed back
           to d-major by PE-transposes whose "identity" operand is
           beta*s*I, fusing the scalar application for free.
"""

import math

import numpy as np

import concourse.bass as bass
import concourse.bacc as bacc
import concourse.tile as tile
from concourse import mybir
from concourse.bass_utils import run_bass_kernel_spmd

F32 = mybir.dt.float32
BF16 = mybir.dt.bfloat16
FP8 = mybir.dt.float8e4

P = 128          # partitions
D = 1024         # model dim
NDT = D // P     # d-tiles (8)
T_FULL = 512
B_FULL = 32
NC = 8           # cores
NB = B_FULL // NC  # batch rows per core (4)
EPS = 1e-6
ROWCHUNK = 512   # projection moving-chunk (rows of (t,b))


def build_nc(T_steps: int, alpha: float, beta: float, lp_dtype=BF16, wh_scale: float = 1.0):
    """Build the Bass program (identical SPMD program for all cores)."""
    nc = bacc.Bacc()

    xT = nc.declare_dram_parameter("xT", [D, T_steps, NB], F32, isOutput=False)
    WT = nc.declare_dram_parameter("WT", [D, D], F32, isOutput=False)
    WhT = nc.declare_dram_parameter("WhT", [D, D], lp_dtype, isOutput=False)
    h0T = nc.declare_dram_parameter("h0T", [D, NB], F32, isOutput=False)
    abias = nc.declare_dram_parameter("abias", [D, 1], F32, isOutput=False)
    h_out = nc.declare_dram_parameter("h_out", [T_steps + 1, P, NDT, NB], F32, isOutput=True)
    outs_out = nc.declare_dram_parameter("outs_out", [T_steps, P, NDT, NB], F32, isOutput=True)

    rows = T_steps * NB

    with tile.TileContext(nc) as tc:
        with (
            tc.tile_pool(name="slabs", bufs=1) as slabs,
            tc.tile_pool(name="projw", bufs=1) as projw,
            tc.tile_pool(name="projx", bufs=2) as projx,
            tc.tile_pool(name="projps", bufs=2, space="PSUM") as projps,
        ):
            # persistent SBUF slabs
            wh_slab = slabs.tile([P, NDT, D], lp_dtype)
            WhT_view = WhT.rearrange("(k p) e -> p k e", p=P)
            for kt in range(NDT):
                nc.sync.dma_start(out=wh_slab[:, kt, :], in_=WhT_view[:, kt, :])
            uT_slab = slabs.tile([P, T_steps, NDT, NB], F32)
            abias_sb = slabs.tile([P, NDT], F32)
            nc.sync.dma_start(out=abias_sb, in_=abias.rearrange("(k p) one -> p (k one)", p=P))

            # scan constants (init before the DMA-heavy projection so these
            # memsets don't inherit WAR waits against all 8 DMA queues)
            ones_col = slabs.tile([P, 1], F32)
            nc.vector.memset(ones_col, 1.0)
            ones_row = slabs.tile([1, P], F32)
            nc.vector.memset(ones_row, 1.0)
            one_1x1 = slabs.tile([1, 1], F32)
            nc.vector.memset(one_1x1, 1.0)
            eps_ap = slabs.tile([1, 1], F32)
            nc.vector.memset(eps_ap, EPS)
            prime_d = slabs.tile([P, 1], F32)
            nc.vector.tensor_copy(prime_d, abias_sb[:, 0:1])  # DVE observes abias DMA
            ident_lp = slabs.tile([NB, NB], BF16)
            nc.vector.memset(ident_lp, 0.0)
            nc.gpsimd.affine_select(
                out=ident_lp, in_=ident_lp,
                compare_op=mybir.AluOpType.not_equal,
                fill=1.0,
                base=0, pattern=[[-1, NB]], channel_multiplier=1,
            )

            # ---------------- Phase A: projection  uT = alpha*(x @ W.T + b) ----------------
            # projection SBUF pools stay open for the whole kernel: reusing
            # their DMA-written SBUF would attach WAR waits against all 8 DMA
            # queues to later instructions (walrus wait-count limit).
            if True:
                wt_slab = projw.tile([P, NDT, D], F32)
                WT_view = WT.rearrange("(k p) e -> p k e", p=P)
                for kt in range(NDT):
                    nc.sync.dma_start(out=wt_slab[:, kt, :], in_=WT_view[:, kt, :])

                xT_view = xT.rearrange("(k p) t b -> p k (t b)", p=P)
                off = 0
                while off < rows:
                    rc = min(ROWCHUNK, rows - off)
                    t0, nt = off // NB, rc // NB
                    xt = projx.tile([P, NDT, rc], F32, tag="xt")
                    for kt in range(NDT):
                        nc.sync.dma_start(out=xt[:, kt, :], in_=xT_view[:, kt, off:off + rc])
                    for et in range(NDT):
                        ps = projps.tile([P, rc], F32, tag="ps")
                        if off >= 2 * ROWCHUNK or et >= 2:
                            # dummy PE write into the slot: funnels the WAR
                            # edge (vs the DVE uT-copy that read this slot)
                            # into one PE instruction with a single wait
                            prev_t0 = max(0, t0 - (2 * ROWCHUNK) // NB) if et < 2 else t0
                            nc.tensor.transpose(
                                ps[0:NDT, 0:1],
                                uT_slab[0:1, prev_t0, :, 0],
                                one_1x1,
                            )
                        for kt in range(NDT):
                            nc.tensor.matmul(
                                ps,
                                wt_slab[:, kt, et * P:(et + 1) * P],
                                xt[:, kt, :],
                                start=(kt == 0),
                                stop=(kt == NDT - 1),
                            )
                        # uT[:, t0:t0+nt, et, :] = alpha*ps + abias[et]  (DVE:
                        # the ACT struct allows only one semaphore wait)
                        nc.vector.tensor_scalar(
                            out=uT_slab[:, t0:t0 + nt, et, :],
                            in0=ps.rearrange("p (t b) -> p t b", b=NB),
                            scalar1=float(alpha),
                            scalar2=abias_sb[:, et:et + 1],
                            op0=mybir.AluOpType.mult,
                            op1=mybir.AluOpType.add,
                        )
                    off += rc

            # ---------------- Phase B: the scan ----------------
            with (
                tc.tile_pool(name="state", bufs=2) as state,
                tc.tile_pool(name="work", bufs=2) as work,
                tc.tile_pool(name="mmps", bufs=1, space="PSUM") as mmps_pool,
                tc.tile_pool(name="mmtps", bufs=2, space="PSUM") as mmt_pool,
                tc.tile_pool(name="smallps", bufs=2, space="PSUM") as small_ps,
            ):
                use_fp8 = lp_dtype == FP8
                lp_cols = 16 if use_fp8 else NB
                # initial state: pre_0 = h0, s_0 = 1
                hT_prev = state.tile([P, NDT, NB], F32, tag="hT")
                nc.sync.dma_start(out=hT_prev, in_=h0T.rearrange("(k p) b -> p k b", p=P))
                pre_lp_prev = state.tile([P, NDT, lp_cols], lp_dtype, tag="prelp")
                nc.scalar.copy(pre_lp_prev[:, :, 0:NB], hT_prev)
                scol_prev = state.tile([NB, 1], F32, tag="scol")
                nc.scalar.activation(
                    scol_prev, ones_col[0:NB, :],
                    mybir.ActivationFunctionType.Copy, scale=1.0 / wh_scale,
                )  # s_0 = 1
                nc.sync.dma_start(out=h_out[0], in_=hT_prev)

                NH = D // 512  # moving chunks of the recurrent matmul (2)
                for k in range(1, T_steps + 1):
                    # (1) mm = pre_{k-1} @ Wh^T   [NB, D] in PSUM (b-major)
                    mm_ps = mmps_pool.tile([NB, D], F32, tag="mm")
                    if use_fp8:
                        # DoubleRow: contraction 256 per matmul over jt-pairs
                        for nch in range(NH):
                            for kt in range(NDT // 2):
                                nc.tensor.matmul(
                                    mm_ps[:, nch * 512:(nch + 1) * 512],
                                    pre_lp_prev[:, 2 * kt:2 * kt + 2, 0:NB],
                                    wh_slab[:, 2 * kt:2 * kt + 2, nch * 512:(nch + 1) * 512],
                                    start=(kt == 0),
                                    stop=(kt == NDT // 2 - 1),
                                    perf_mode=mybir.MatmulPerfMode.DoubleRow,
                                )
                    else:
                        for nch in range(NH):
                            for kt in range(NDT):
                                nc.tensor.matmul(
                                    mm_ps[:, nch * 512:(nch + 1) * 512],
                                    pre_lp_prev[:, kt, :],
                                    wh_slab[:, kt, nch * 512:(nch + 1) * 512],
                                    start=(kt == 0),
                                    stop=(kt == NDT - 1),
                                )
                    # (2) cast to bf16 in SBUF (per 512-chunk, overlaps the stream)
                    mm_sb = work.tile([NB, NDT, P], BF16, tag="mmsb")
                    for nch in range(NH):
                        nc.scalar.activation(
                            out=mm_sb[:, nch * 4:(nch + 1) * 4, :],
                            in_=mm_ps[:, nch * 512:(nch + 1) * 512].rearrange(
                                "p (a c) -> p a c", c=P),
                            func=mybir.ActivationFunctionType.Copy,
                            scale=scol_prev,
                        )
                    # (3) transpose back to d-major, scaled by beta*s_{k-1}
                    mmT_ps = mmt_pool.tile([P, NDT, NB], BF16, tag="mmt")
                    for jt in range(NDT):
                        nc.tensor.transpose(mmT_ps[:, jt, :], mm_sb[:, jt, :], ident_lp)
                    # (4) pre_k = h_{k-1} + u_k + mmT
                    tmp = work.tile([P, NDT, NB], F32, tag="tmp")
                    nc.vector.tensor_add(tmp, hT_prev, uT_slab[:, k - 1, :, :])
                    preT = work.tile([P, NDT, NB], F32, tag="pre")
                    nc.vector.tensor_add(preT, tmp, mmT_ps)
                    # (5) cast for the next matmul (critical path!)
                    pre_lp = state.tile([P, NDT, lp_cols], lp_dtype, tag="prelp")
                    nc.scalar.copy(pre_lp[:, :, 0:NB], preT)

                    # ---- norm branch (off critical path) ----
                    sq = work.tile([P, NDT, NB], F32, tag="sq")
                    nc.vector.tensor_mul(sq, preT, preT)
                    ssq_ps = small_ps.tile([1, NDT, NB], F32, tag="small")
                    nc.tensor.matmul(
                        ssq_ps.rearrange("p a b -> p (a b)"),
                        ones_col,
                        sq.rearrange("p a b -> p (a b)"),
                        start=True, stop=True,
                    )
                    ssq_r = work.tile([1, NB], F32, tag="ssqr")
                    nc.vector.tensor_reduce(
                        ssq_r, ssq_ps.rearrange("p a b -> p b a"),
                        axis=mybir.AxisListType.X, op=mybir.AluOpType.add,
                    )
                    s_tmp = work.tile([1, NB], F32, tag="stmp")
                    nc.scalar.activation(
                        s_tmp, ssq_r, mybir.ActivationFunctionType.Sqrt,
                        bias=eps_ap, scale=1.0 / D,
                    )
                    s_row = work.tile([1, NB], F32, tag="srow")
                    nc.vector.reciprocal(s_row, s_tmp)
                    # replicate s_row 8x along free for the broadcast matmul
                    srep = work.tile([1, NDT, NB], F32, tag="srep")
                    nc.vector.tensor_copy(
                        out=srep,
                        in_=bass.AP(
                            tensor=s_row.tensor, offset=s_row.offset,
                            ap=[s_row.ap[0], [0, NDT], s_row.ap[-1]],
                        ),
                    )
                    # s as a column (for the diag build)
                    scol_ps = small_ps.tile([NB, 1], F32, tag="small")
                    nc.tensor.transpose(scol_ps, s_row, one_1x1)
                    scol = state.tile([NB, 1], F32, tag="scol")
                    nc.scalar.activation(
                        scol, scol_ps, mybir.ActivationFunctionType.Copy,
                        scale=1.0 / wh_scale,
                    )
                    # broadcast s across partitions: sbc = ones_col128 x srep
                    sbc_ps = small_ps.tile([P, NDT, NB], F32, tag="small")
                    nc.tensor.matmul(
                        sbc_ps.rearrange("p a b -> p (a b)"),
                        ones_row,
                        srep.rearrange("p a b -> p (a b)"),
                        start=True, stop=True,
                    )
                    # h_{k} ... wait: h at THIS step uses s_k on pre_k
                    hT = state.tile([P, NDT, NB], F32, tag="hT")
                    nc.vector.tensor_mul(hT, preT, sbc_ps)
                    sg = work.tile([P, NDT, NB], F32, tag="sg")
                    nc.scalar.activation(sg, hT, mybir.ActivationFunctionType.Sigmoid)
                    hsg = work.tile([P, NDT, NB], F32, tag="hsg")
                    nc.vector.tensor_mul(hsg, hT, sg)
                    outT = work.tile([P, NDT, NB], F32, tag="outT")
                    nc.vector.tensor_mul(outT, hsg, hT)
                    nc.sync.dma_start(out=h_out[k], in_=hT)
                    nc.sync.dma_start(out=outs_out[k - 1], in_=outT)

                    hT_prev, pre_lp_prev, scol_prev = hT, pre_lp, scol

    nc.compile()  # bacc legalization: ≤1 sync wait per instruction
    return nc


_NC_CACHE: dict = {}


def _get_nc(key, *args, **kwargs):
    if key not in _NC_CACHE:
        _NC_CACHE[key] = build_nc(*args, **kwargs)
    return _NC_CACHE[key]


def prepare_in_maps(x, h0, W, W_h, b, alpha, beta, lp_np=None, wh_scale: float = 1.0):
    """Host-side shard + transpose prep. Returns list of per-core input dicts."""
    import ml_dtypes
    if lp_np is None:
        lp_np = ml_dtypes.bfloat16
    T_steps = x.shape[0]
    WT_h = np.ascontiguousarray(W.T.astype(np.float32))
    WhT_h = np.ascontiguousarray((W_h.T * (beta * wh_scale)).astype(lp_np))
    abias_h = np.ascontiguousarray((alpha * b).astype(np.float32).reshape(D, 1))
    in_maps = []
    for c in range(NC):
        sl = slice(c * NB, (c + 1) * NB)
        xT_h = np.ascontiguousarray(x[:, sl, :].transpose(2, 0, 1).astype(np.float32))
        h0T_h = np.ascontiguousarray(h0[sl].T.astype(np.float32))
        in_maps.append({
            "xT": xT_h, "WT": WT_h, "WhT": WhT_h, "h0T": h0T_h, "abias": abias_h,
        })
    return in_maps


def assemble(results, T_steps):
    outs = np.empty((T_steps, B_FULL, D), np.float32)
    h = np.empty((T_steps + 1, B_FULL, D), np.float32)
    for c, r in enumerate(results):
        sl = slice(c * NB, (c + 1) * NB)
        outs[:, sl, :] = r["outs_out"].transpose(0, 3, 2, 1).reshape(T_steps, NB, D)
        h[:, sl, :] = r["h_out"].transpose(0, 3, 2, 1).reshape(T_steps + 1, NB, D)
    return outs, h


def run(x, h0, W, W_h, b, log_alpha, log_beta, trace=False, use_fp8=True):
    x = np.asarray(x, np.float32)
    h0 = np.asarray(h0, np.float32)
    W = np.asarray(W, np.float32)
    W_h = np.asarray(W_h, np.float32)
    b = np.asarray(b, np.float32)
    alpha = float(np.exp(np.float32(log_alpha)))
    beta = float(1.0 / (1.0 + math.exp(-float(log_beta))) * 0.1)
    T_steps = x.shape[0]

    import ml_dtypes
    if use_fp8:
        nc = _get_nc(("fp8", T_steps, alpha, beta), T_steps, alpha, beta,
                     lp_dtype=FP8, wh_scale=4096.0)
        in_maps = prepare_in_maps(x, h0, W, W_h, b, alpha, beta,
                                  lp_np=ml_dtypes.float8_e4m3, wh_scale=4096.0)
    else:
        nc = _get_nc(("v1", T_steps, alpha, beta), T_steps, alpha, beta)
        in_maps = prepare_in_maps(x, h0, W, W_h, b, alpha, beta)
    res = run_bass_kernel_spmd(nc, in_maps, list(range(NC)), trace=trace)
    outs, h = assemble(res.results, T_steps)
    return outs, h, res


def kernel(x, h0, W, W_h, b, log_alpha, log_beta):
    outs, h, _ = run(x, h0, W, W_h, b, log_alpha, log_beta)
    return outs, h
